# revision 26
# baseline (speedup 1.0000x reference)
"""Bidirectional LSTM (S=2048, B=4096, I=1, H=8, O=1) on 8 Trainium2 NeuronCores.

Strategy (v2)
-------------
Pure data parallel over batch (512 rows/core) plus sequence chunking with
warmup: a chunk started W steps early from zero state converges to the true
trajectory (forget-gate contraction ~0.6/step) before its first emitted
output.

Per core: G=3 pipelined groups x NP=7 chunk-stream pairs (fwd+bwd), chunk
length l=98.  The 7 (fwd,bwd) pairs of a group are stacked block-diagonally:
rhs = [h (112 rows) ; x (14) ; ones (1)] = [127, 512] fp16.

All activations are SIGMOID (one ACT table, maximal merging):
  tanh(x) = 2*sigmoid(2x) - 1
  - g-gate: stationary weights pre-scaled 2x -> psum holds 2*g~;
    g = 2*sig(2g~)-1 folded into DVE scalar_tensor_tensor ops:
       z  = (sig2g - 0.5) * i          [= i*g/2]
       c' = (z * 2) + f*c
  - tanh(c): ACT sigmoid with free scale=2.0; h/2 = (sig2c - 0.5) * o is the
    STORED state, with the 2x folded into the h-columns of all stationaries.

Per group-round (one step of 14 streams):
  PE : 4 matmuls [127x119/112]@[127x512] fp16 -> one psum tile [128,4,512]
       (4 banks); the f-gate stationary carries 7 extra columns computing
       w_out . h for the PREVIOUS step into psum partitions 112..118.
  ACT: ONE merged sigmoid over [112,4,512] (all gates) + sigmoid(2c).
  DVE: f*c (TT) + 3 fused scalar_tensor_tensor ops; h' written straight
       into the next rhs tile (fp16 => 2x DVE mode).
  DMA: next x rows into rhs; out rows [7,512] psum -> HBM (b_out added host
       side).

PSUM = 2 figo tiles x 4 banks = 8 banks, rotating across the 3 groups.
"""

import os
import sys

if "axon" not in os.environ.get("JAX_PLATFORMS", "axon"):
    os.environ["JAX_PLATFORMS"] = "axon,cpu"

try:
    import concourse  # noqa: F401
except ImportError:  # pragma: no cover
    sys.path.insert(0, "/opt/trn_rl_repo")

from contextlib import ExitStack

import numpy as np

import concourse.bacc as bacc
import concourse.mybir as mybir
import concourse.tile as tile

S, B, I, H, O = 2048, 4096, 1, 8, 1
N_CORES = 8
BC = B // N_CORES

NP = 7   # stream pairs per group
G = 3    # pipelined groups per core
W = 8    # warmup rounds per chunk

KH = 16 * NP          # 112 h rows / gate partitions
KR = KH + 2 * NP + 1  # 127 rhs rows (h + x + ones)

GATES = ("f", "i", "g", "o")
TORCH_BLOCK = {"i": 0, "f": 1, "g": 2, "o": 3}

F32 = mybir.dt.float32
F16 = mybir.dt.float16
AF = mybir.ActivationFunctionType
ALU = mybir.AluOpType


def _lchunk():
    return -(-S // (NP * G))  # ceil; tail chunk padded with zero x


# --------------------------------------------------------------------------
# host-side data preparation
# --------------------------------------------------------------------------

def make_weights(wihs, whhs, bihs, bhhs, w_out):
    """Stationary operands [KR, 119|112] fp16.

    Columns 16s+8d..+8 = gate rows of pair s, direction d.  h-block entries
    are 2x (stored h = h/2); the whole g-gate stationary is an extra 2x
    (sigmoid(2*g~) trick).  w_f gets 7 extra columns (112+s) computing
    w_out . h of the step held in rhs.
    """
    out = {}
    for q in GATES:
        bi = TORCH_BLOCK[q]
        sc = 2.0 if q == "g" else 1.0
        w = np.zeros((KR, 119), np.float32)
        for s in range(NP):
            for d in range(2):
                c0 = 16 * s + 8 * d
                w[c0:c0 + 8, c0:c0 + 8] = sc * whhs[d][8 * bi:8 * bi + 8, :].T
                w[KH + 2 * s + d, c0:c0 + 8] = sc * wihs[d][8 * bi:8 * bi + 8, 0]
                w[KR - 1, c0:c0 + 8] = sc * (bihs[d] + bhhs[d])[8 * bi:8 * bi + 8]
        if q == "f":
            # out columns: psum = sum(w_out/2 * h) = (w_out.h)/2, so the
            # merged sigmoid emits sig(out/2); host applies 2*logit
            for s in range(NP):
                for d in range(2):
                    c0 = 16 * s + 8 * d
                    w[c0:c0 + 8, 112 + s] = 0.5 * w_out[0, 8 * d:8 * d + 8]
        wm = w.copy()
        wm[:, 0:16] = 0.0  # zero pair-0 gate cols -> chunk-0 state pinned to 0
        out[f"w_{q}"] = w.astype(np.float16)
        out[f"w_{q}_warm"] = wm.astype(np.float16)
    # one packed tensor -> one startup DMA instead of 8 serialized ones
    order = [f"w_{q}{v}" for q in GATES for v in ("", "_warm")]
    return {"w_all": np.stack([out[k] for k in order], axis=1)}


def make_xarr(x_core, future):
    """Per-core x arranged as [G, R+1, 15, bc] fp16; row 14 is ones (bias)."""
    l_chunk = _lchunk()
    R = l_chunk + W
    s_len, bc = x_core.shape
    xb = x_core[(future - np.arange(s_len)) % s_len]
    xa = np.zeros((G, R + 1, 2 * NP + 1, bc), np.float32)
    xa[:, :, 2 * NP, :] = 1.0
    rr = np.arange(R + 1)
    for g in range(G):
        for s in range(NP):
            pos = (g * NP + s) * l_chunk - W + rr
            valid = (pos >= 0) & (pos < s_len)
            for d, src in enumerate((x_core, xb)):
                xa[g, valid, 2 * s + d, :] = src[pos[valid]]
    return xa.astype(np.float16)


def make_in_maps(x, wihs, whhs, bihs, bhhs, w_out, b_out, future):
    shared = make_weights(wihs, whhs, bihs, bhhs, w_out)
    in_maps = []
    for k in range(N_CORES):
        m = dict(shared)
        m["xarr"] = make_xarr(x[:, k * BC:(k + 1) * BC, 0], future)
        in_maps.append(m)
    return in_maps


# --------------------------------------------------------------------------
# program builder
# --------------------------------------------------------------------------

def build_program(num_devices=N_CORES):
    l_chunk = _lchunk()
    R = l_chunk + W
    s_pad = l_chunk * NP * G

    nc = bacc.Bacc("TRN2", target_bir_lowering=False, debug=False,
                   enable_asserts=False, num_devices=num_devices)

    dram = {}
    host_names = []

    def din(name, shape, dt_=F16):
        dram[name] = nc.dram_tensor(name, list(shape), dt_, kind="ExternalInput").ap()
        host_names.append(name)

    din("w_all", (KR, 8, 119))
    din("xarr", (G, R + 1, 2 * NP + 1, BC))
    out_d = nc.dram_tensor("out", [s_pad, BC], F16, kind="ExternalOutput").ap()
    out_view = out_d.rearrange("(c l) b -> c l b", l=l_chunk)

    with tile.TileContext(nc) as tc, ExitStack() as ctx:
        consts = ctx.enter_context(tc.tile_pool(name="consts", bufs=1))
        rhp = ctx.enter_context(tc.tile_pool(name="rhp", bufs=6))
        up = ctx.enter_context(tc.tile_pool(name="up", bufs=4))
        cp = ctx.enter_context(tc.tile_pool(name="cp", bufs=6))
        tp = ctx.enter_context(tc.tile_pool(name="tp", bufs=4))
        zp = ctx.enter_context(tc.tile_pool(name="zp", bufs=4))
        kp = ctx.enter_context(tc.tile_pool(name="kp", bufs=4))
        fpp = ctx.enter_context(tc.tile_pool(name="fpp", bufs=2, space="PSUM"))

        # trigger the sigmoid/tanh ACT table load at t=0 so the ~2.7us load
        # overlaps the constant DMAs instead of stalling the first real sigmoid
        dum = consts.tile([1, 2], F32, name="dum", tag="dum")
        nc.vector.memset(dum, 0.0)
        nc.scalar.activation(dum[:, 1:2], dum[:, 0:1], AF.Sigmoid)

        w_all = consts.tile([KR, 8, 119], F16, name="c_w_all", tag="c_w_all")
        # issued from the (startup-idle) ACT queue, parallel to xarr on Sync
        nc.scalar.dma_start(out=w_all, in_=dram["w_all"])
        ct = {}
        for j, (q, v) in enumerate((q, v) for q in GATES for v in ("", "_warm")):
            ct[f"w_{q}{v}"] = w_all[:, j, :]

        rhs_cur, c_prev = [], []
        for g in range(G):
            r0 = rhp.tile([KR, BC], F16, name=f"rhs0_{g}", tag="rhs")
            nc.vector.memset(r0[0:KH, :], 0.0)
            nc.sync.dma_start(out=r0[KH:KR, :], in_=dram["xarr"][g, 0])
            c0 = cp.tile([KH, BC], F16, name=f"c0_{g}", tag="c")
            nc.vector.memset(c0, 0.0)
            rhs_cur.append(r0)
            c_prev.append(c0)

        for r in range(R + 1):
            # phase 1: all matmuls (w_f also produces (w_out.h(r-1))/2 in
            # partitions 112..118; the merged sigmoid turns it into
            # sig(out/2) which the host inverts with 2*logit)
            figos = []
            for g in range(G):
                rhs = rhs_cur[g]
                warm = "_warm" if (g == 0 and r < W) else ""
                figo = fpp.tile([128, 4, BC], F32, name=f"ps_{g}_{r}", tag="figo")
                nc.tensor.matmul(figo[0:119, 0, :], ct[f"w_f{warm}"], rhs,
                                 start=True, stop=True)
                if r < R:
                    nc.tensor.matmul(figo[0:119, 1, :], ct[f"w_i{warm}"], rhs,
                                     start=True, stop=True)
                    nc.tensor.matmul(figo[0:119, 2, :], ct[f"w_g{warm}"], rhs,
                                     start=True, stop=True)
                    nc.tensor.matmul(figo[0:119, 3, :], ct[f"w_o{warm}"], rhs,
                                     start=True, stop=True)
                figos.append(figo)

            if r == R:
                for g in range(G):
                    u = up.tile([119, 4, BC], F16, name=f"u_{g}_{r}", tag="u")
                    nc.scalar.activation(u[:, 0, :], figos[g][0:119, 0, :],
                                         AF.Sigmoid)
                    nc.sync.dma_start(
                        out=out_view[g * NP:(g + 1) * NP, r - 1 - W, :],
                        in_=u[112:119, 0, :])
                continue  # final round exists only to flush the last outputs

            # staggered per-group emission: group g's tanh(c)/h' tail is
            # emitted one group-slot later so no engine queue head-blocks
            def finish(g, u, cn, rhs_n):
                tcn = kp.tile([KH, BC], F16, name=f"tc_{g}_{r}", tag="tc")
                nc.scalar.activation(tcn, cn, AF.Tanh)
                nc.vector.tensor_mul(rhs_n[0:KH, :], tcn, u[0:KH, 3, :])
                rhs_cur[g], c_prev[g] = rhs_n, cn

            pend = None
            for g in range(G):
                u = up.tile([119, 4, BC], F16, name=f"u_{g}_{r}", tag="u")
                nc.scalar.activation(u, figos[g][0:119, :, :], AF.Sigmoid)
                if r >= W + 1:
                    nc.sync.dma_start(
                        out=out_view[g * NP:(g + 1) * NP, r - 1 - W, :],
                        in_=u[112:119, 0, :])
                # x rows for the NEXT rhs are DMAed a full slot before the
                # h' write so the DMA-completion sem never gates the matmuls
                rhs_n = rhp.tile([KR, BC], F16, name=f"rhs_{g}_{r}", tag="rhs")
                nc.sync.dma_start(out=rhs_n[KH:KR, :], in_=dram["xarr"][g, r + 1])
                tm = tp.tile([KH, BC], F16, name=f"tm_{g}_{r}", tag="tm")
                nc.vector.tensor_mul(tm, u[0:KH, 0, :], c_prev[g])
                z = zp.tile([KH, BC], F16, name=f"z_{g}_{r}", tag="z")
                nc.vector.scalar_tensor_tensor(
                    z, u[0:KH, 2, :], 0.5, u[0:KH, 1, :], ALU.subtract, ALU.mult)
                cn = cp.tile([KH, BC], F16, name=f"c_{g}_{r}", tag="c")
                nc.vector.scalar_tensor_tensor(
                    cn, z, 2.0, tm, ALU.mult, ALU.add)
                if pend is not None:
                    finish(*pend)
                pend = (g, u, cn, rhs_n)
            finish(*pend)

    nc.compile()
    return nc, host_names


# --------------------------------------------------------------------------
# runner
# --------------------------------------------------------------------------

_CACHE = {}


def _get_program():
    key = (NP, G, W, BC, S)
    if key not in _CACHE:
        _CACHE[key] = build_program()
    return _CACHE[key]


def kernel(x, w_ih_f, w_hh_f, b_ih_f, b_hh_f, w_ih_b, w_hh_b, b_ih_b, b_hh_b,
           w_out, b_out, future):
    from concourse import bass_utils

    x = np.asarray(x, np.float32)
    wihs = [np.asarray(w_ih_f, np.float32), np.asarray(w_ih_b, np.float32)]
    whhs = [np.asarray(w_hh_f, np.float32), np.asarray(w_hh_b, np.float32)]
    bihs = [np.asarray(b_ih_f, np.float32), np.asarray(b_ih_b, np.float32)]
    bhhs = [np.asarray(b_hh_f, np.float32), np.asarray(b_hh_b, np.float32)]
    w_out = np.asarray(w_out, np.float32)
    b_out = float(np.asarray(b_out).reshape(-1)[0])
    future = int(future)

    nc, names = _get_program()
    in_maps = make_in_maps(x, wihs, whhs, bihs, bhhs, w_out, b_out, future)
    res = bass_utils.run_bass_kernel_spmd(nc, in_maps, core_ids=list(range(N_CORES)))
    out = np.empty((B, S), np.float32)
    for k in range(N_CORES):
        u = np.asarray(res.results[k]["out"][:S, :], np.float32).T
        u = np.clip(u, 1e-4, 1.0 - 1e-4)
        out[k * BC:(k + 1) * BC, :] = 2.0 * np.log(u / (1.0 - u))
    out += b_out
    return out
